# revision 1
# baseline (speedup 1.0000x reference)
"""Trainium2 Bass kernel for JetMoE MoE layer (nn_JetMoeMoE).

Expert-parallel dense MoE across 8 NeuronCores:
  - core e holds expert e's weights (w_in[e], w_out[e])
  - every core computes the router (f32 matmul + top-2 softmax gates) for
    all T=4096 tokens on-device, then selects its own expert's gate column
    via a one-hot `sel` input (keeps the program identical across cores)
  - every core computes its expert's full GLU MLP for all tokens in bf16
    and writes the gate-weighted partial output [T, D] (+ bias/8 so the
    partials sum to acc + bias)
  - host sums the 8 partials (the unshard step for expert-parallel)

Shapes (hardcoded): B=2, L=2048 -> T=4096 tokens, D=2048, H=4096, E=8.
reference: h = x @ wi.T; glu = silu(h[:, :H]) * h[:, H:]; o = glu @ wo.T;
           out = sum_e gate_e * o_e + bias.
Gates: top-2 softmax over router logits == sigmoid of the logit gap,
       computed exactly (f32 logits) so expert selection matches the
       reference.
"""

import sys

sys.path.insert(0, "/opt/trn_rl_repo")

import numpy as np
import ml_dtypes

import concourse.bass as bass
import concourse.mybir as mybir
import concourse.tile as tile
from concourse import bacc
from concourse.bass_utils import run_bass_kernel_spmd

F32 = mybir.dt.float32
BF16 = mybir.dt.bfloat16
AX = mybir.AxisListType.X
OP = mybir.AluOpType
ACTF = mybir.ActivationFunctionType

P = 128
D = 2048
H = 4096
T = 4096
E = 8
TB = 512          # tokens per block
NBLK = T // TB    # 8 blocks
NTS = TB // P     # 4 token subtiles per block
NK1 = D // P      # 16 contraction tiles for router / h matmul
NJ = H // P       # 32 GLU feature tiles (each pairs f-tile j with j+NJ)
NK2 = H // P      # 32 contraction tiles for down-proj
ND = D // 512     # 4 output d-tiles of 512


def _emit_moe(tc, xtr, xtb, rw, sel, wi, wo, b8, out,
              reps=1, do_router=True, do_phase2=True):
    nc = tc.nc
    with (
        tc.tile_pool(name="const", bufs=1) as constp,
        tc.tile_pool(name="xr", bufs=2) as xrp,
        tc.tile_pool(name="xb", bufs=2) as xbp,
        tc.tile_pool(name="wip", bufs=3) as wip,
        tc.tile_pool(name="wop", bufs=3) as wop,
        tc.tile_pool(name="glup", bufs=2) as glup,
        tc.tile_pool(name="gatep", bufs=8) as gatep,
        tc.tile_pool(name="tmpp", bufs=3) as tmpp,
        tc.tile_pool(name="outp", bufs=3) as outp,
        tc.tile_pool(name="ps_r", bufs=2, space="PSUM") as ps_r,
        tc.tile_pool(name="ps_h", bufs=2, space="PSUM") as ps_h,
        tc.tile_pool(name="ps_o", bufs=2, space="PSUM") as ps_o,
    ):
        rw_t = constp.tile([P, NK1, E], F32)
        nc.sync.dma_start(rw_t[:], rw[:])
        sel_t = constp.tile([P, E], F32)
        nc.sync.dma_start(sel_t[:], sel[:])
        b8_t = constp.tile([P, D], F32)
        nc.sync.dma_start(b8_t[:], b8[:])
        # bias/8: the 8 cores each add bias/8 so the summed partials carry bias
        nc.scalar.mul(b8_t[:], b8_t[:], 0.125)

        nblk = out.shape[0] // TB // reps
        for rep in range(reps):
          for blk in range(nblk):
            oblk = rep * nblk + blk
            # ---- router: f32 logits + top-2 softmax gates for this block
            ge_ts = []
            for ts_ in range(NTS):
                if not do_router:
                    ge_t = gatep.tile([P, 1], F32)
                    nc.vector.memset(ge_t[:], 0.5)
                    ge_ts.append(ge_t)
                    continue
                xr_t = xrp.tile([P, NK1, P], F32)
                nc.sync.dma_start(xr_t[:], xtr[blk * NTS + ts_])
                lg = ps_r.tile([P, E], F32)
                for k in range(NK1):
                    nc.tensor.matmul(
                        lg[:], xr_t[:, k, :], rw_t[:, k, :],
                        start=(k == 0), stop=(k == NK1 - 1),
                    )
                v1 = tmpp.tile([P, 1], F32)
                nc.vector.reduce_max(v1[:], lg[:], axis=AX)
                is_top = tmpp.tile([P, E], F32)
                nc.vector.tensor_scalar(is_top[:], lg[:], v1[:], None, OP.is_ge)
                # mask out argmax entries, then re-max for the 2nd-best value
                masked = tmpp.tile([P, E], F32)
                nc.vector.scalar_tensor_tensor(
                    masked[:], is_top[:], -1.0e30, lg[:], OP.mult, OP.add
                )
                v2 = tmpp.tile([P, 1], F32)
                nc.vector.reduce_max(v2[:], masked[:], axis=AX)
                dgap = tmpp.tile([P, 1], F32)
                nc.vector.tensor_sub(dgap[:], v1[:], v2[:])
                g1 = tmpp.tile([P, 1], F32)
                nc.scalar.activation(g1[:], dgap[:], ACTF.Sigmoid)
                # g2 = 1 - g1 ; d12 = g1 - g2 = 2*g1 - 1
                g2 = tmpp.tile([P, 1], F32)
                nc.vector.tensor_scalar(g2[:], g1[:], -1.0, 1.0, OP.mult, OP.add)
                d12 = tmpp.tile([P, 1], F32)
                nc.vector.tensor_scalar(d12[:], g1[:], 2.0, -1.0, OP.mult, OP.add)
                is_2nd = tmpp.tile([P, E], F32)
                nc.vector.tensor_scalar(is_2nd[:], lg[:], v2[:], None, OP.is_ge)
                # gates8 = is_top*(g1-g2) + is_2nd*g2  (is_top implies is_2nd)
                gates8 = tmpp.tile([P, E], F32)
                nc.vector.tensor_scalar(gates8[:], is_top[:], d12[:], None, OP.mult)
                g8b = tmpp.tile([P, E], F32)
                nc.vector.tensor_scalar(g8b[:], is_2nd[:], g2[:], None, OP.mult)
                nc.vector.tensor_add(gates8[:], gates8[:], g8b[:])
                # pick this core's expert column via the one-hot sel input
                nc.vector.tensor_mul(gates8[:], gates8[:], sel_t[:])
                ge_t = gatep.tile([P, 1], F32)
                nc.vector.reduce_sum(ge_t[:], gates8[:], axis=AX)
                ge_ts.append(ge_t)

            # ---- phase 1: hT = wiT.T @ xT, GLU -> gluT [H, TB] bf16
            xb_t = xbp.tile([P, NK1, TB], BF16)
            # split across DMA queues (one dma_start ~31 GB/s per queue)
            for c in range(4):
                nc.sync.dma_start(
                    xb_t[:, c * 4:(c + 1) * 4, :], xtb[blk, :, c * 4:(c + 1) * 4, :]
                )
            glu_t = glup.tile([P, NJ, TB], BF16)
            for j in range(NJ):
                wia = wip.tile([P, NK1, P], BF16)
                wib = wip.tile([P, NK1, P], BF16)
                for c in range(2):
                    ks = slice(c * NK1 // 2, (c + 1) * NK1 // 2)
                    nc.sync.dma_start(wia[:, ks, :], wi[j, :, ks, :])
                    nc.sync.dma_start(wib[:, ks, :], wi[j + NJ, :, ks, :])
                pa = ps_h.tile([P, TB], F32)
                pb = ps_h.tile([P, TB], F32)
                for k in range(NK1):
                    nc.tensor.matmul(
                        pa[:], wia[:, k, :], xb_t[:, k, :],
                        start=(k == 0), stop=(k == NK1 - 1),
                    )
                for k in range(NK1):
                    nc.tensor.matmul(
                        pb[:], wib[:, k, :], xb_t[:, k, :],
                        start=(k == 0), stop=(k == NK1 - 1),
                    )
                sa = tmpp.tile([P, TB], F32, bufs=2)
                nc.scalar.activation(sa[:], pa[:], ACTF.Silu)
                nc.vector.tensor_mul(glu_t[:, j, :], sa[:], pb[:])

            # ---- phase 2: o = gluT.T @ woT, gate, +bias/8, store
            if not do_phase2:
                ob = outp.tile([P, 512], F32)
                nc.vector.tensor_copy(ob[:], glu_t[:, 0, :])
                nc.sync.dma_start(out[oblk * TB:oblk * TB + P, 0:512], ob[:])
                continue
            for n in range(ND):
                # wo tile split in k-halves to fit SBUF (16KB/partition each)
                wo_c0 = wop.tile([P, NK2 // 2, 512], BF16, tag="wo")
                wo_c1 = wop.tile([P, NK2 // 2, 512], BF16, tag="wo")
                for c in range(2):
                    ks = slice(c * NK2 // 4, (c + 1) * NK2 // 4)
                    nc.sync.dma_start(wo_c0[:, ks, :], wo[n, :, ks, :])
                    nc.sync.dma_start(
                        wo_c1[:, ks, :],
                        wo[n, :, NK2 // 2 + c * NK2 // 4: NK2 // 2 + (c + 1) * NK2 // 4, :],
                    )
                for ts_ in range(NTS):
                    po = ps_o.tile([P, 512], F32)
                    for k in range(NK2):
                        wo_c = wo_c0 if k < NK2 // 2 else wo_c1
                        nc.tensor.matmul(
                            po[:],
                            glu_t[:, k, ts_ * P:(ts_ + 1) * P],
                            wo_c[:, k % (NK2 // 2), :],
                            start=(k == 0), stop=(k == NK2 - 1),
                        )
                    ob = outp.tile([P, 512], F32)
                    nc.vector.scalar_tensor_tensor(
                        ob[:], po[:], ge_ts[ts_][:],
                        b8_t[:, n * 512:(n + 1) * 512],
                        OP.mult, OP.add,
                    )
                    r0 = oblk * TB + ts_ * P
                    nc.sync.dma_start(
                        out[r0:r0 + P, n * 512:(n + 1) * 512], ob[:]
                    )


_NC_CACHE = {}


def _get_nc(nblk=NBLK, reps=1, do_router=True, do_phase2=True):
    key = (nblk, reps, do_router, do_phase2)
    if key in _NC_CACHE:
        return _NC_CACHE[key]
    t = nblk * TB * reps
    nc = bacc.Bacc("TRN2", target_bir_lowering=False, debug=False, num_devices=8)
    xtr = nc.dram_tensor("xtr", [nblk * NTS, P, NK1, P], F32, kind="ExternalInput")
    xtb = nc.dram_tensor("xtb", [nblk, P, NK1, TB], BF16, kind="ExternalInput")
    rw = nc.dram_tensor("rw", [P, NK1, E], F32, kind="ExternalInput")
    sel = nc.dram_tensor("sel", [P, E], F32, kind="ExternalInput")
    wi = nc.dram_tensor("wi", [2 * NJ, P, NK1, P], BF16, kind="ExternalInput")
    wo = nc.dram_tensor("wo", [ND, P, NK2, 512], BF16, kind="ExternalInput")
    b8 = nc.dram_tensor("b8", [P, D], F32, kind="ExternalInput")
    out = nc.dram_tensor("out", [t, D], F32, kind="ExternalOutput")
    with tile.TileContext(nc) as tc:
        _emit_moe(tc, xtr.ap(), xtb.ap(), rw.ap(), sel.ap(), wi.ap(),
                  wo.ap(), b8.ap(), out.ap(),
                  reps=reps, do_router=do_router, do_phase2=do_phase2)
    nc.compile()
    _NC_CACHE[key] = nc
    return nc


def _shard_inputs(layer_input, router_weight, w_in, w_out, bias, nblk=NBLK):
    """Host-side shard/layout prep. Returns the 8 per-core input maps."""
    t = nblk * TB
    x = np.ascontiguousarray(np.asarray(layer_input, np.float32).reshape(-1, D))[:t]
    bf = ml_dtypes.bfloat16

    # x in [token, d]; device wants xT tiles [p(d-lo), k(d-hi), t(tok-lo)]
    xtr = np.ascontiguousarray(
        x.reshape(t // P, P, NK1, P).transpose(0, 3, 2, 1))
    xtb = np.ascontiguousarray(
        x.reshape(nblk, TB, NK1, P).transpose(0, 3, 2, 1).astype(bf))
    rw = np.ascontiguousarray(
        np.asarray(router_weight, np.float32).reshape(E, NK1, P).transpose(2, 1, 0))
    b8 = np.ascontiguousarray(
        np.broadcast_to(np.asarray(bias, np.float32), (P, D)))

    in_maps = []
    for e in range(E):
        wi_e = np.asarray(w_in[e], np.float32)   # [2H, D]
        wo_e = np.asarray(w_out[e], np.float32)  # [D, H]
        wi_r = np.ascontiguousarray(
            wi_e.reshape(2 * NJ, P, NK1, P).transpose(0, 3, 2, 1).astype(bf))
        wo_r = np.ascontiguousarray(
            wo_e.reshape(ND, 512, NK2, P).transpose(0, 3, 2, 1).astype(bf))
        onehot = np.zeros((E,), np.float32)
        onehot[e] = 1.0
        sel = np.ascontiguousarray(np.broadcast_to(onehot, (P, E)))
        in_maps.append({
            "xtr": xtr, "xtb": xtb, "rw": rw, "sel": sel,
            "wi": wi_r, "wo": wo_r, "b8": b8,
        })
    return in_maps


def kernel(layer_input, router_weight, w_in, w_out, bias, _nblk=NBLK, _trace=False):
    B, L, _ = np.asarray(layer_input).shape
    nc = _get_nc(_nblk)
    in_maps = _shard_inputs(layer_input, router_weight, w_in, w_out, bias, _nblk)
    res = run_bass_kernel_spmd(nc, in_maps, core_ids=list(range(8)), trace=_trace)
    acc = np.zeros((_nblk * TB, D), np.float64)
    for c in range(8):
        acc += res.results[c]["out"].astype(np.float64)
    full = acc.astype(np.float32)
    if _trace:
        kernel._last_results = res
    if _nblk == NBLK:
        return full.reshape(B, L, D)
    return full

